# revision 1
# baseline (speedup 1.0000x reference)
"""Trainium2 Bass kernel for CandidateFinder (retrieval_knn).

Math: for each (batch, query row), candidates = the K_MAX=64 smallest key
indices whose 32-dim sign pattern matches the query's in either dim-group
(dims 0:32 or 32:64), ascending, padded with -1.  This equals the
reference's per-group-topk + merge (per-group truncation can never drop an
index that would make the merged top-64).

Host side (sharding/layout only, no arithmetic): batch b and query-half
go to core 2b+half; queries are laid out dim-major with pairs of 128-row
slabs stacked into the four 32-partition PE strips, and keys are laid out
dim-major and replicated onto the upper 64 partitions, so four K=32
matmuls can run concurrently in PE row-groups (tile_position).

Device per core:
  - sign-quantize to {+0.5,-0.5} bf16 on DVE (x>0 exactly as reference)
  - all-pairs group dots (match <=> dot == 8); detection units split
    across DVE (row-max) and ACT (relu-sum), both thresholded at 7.9
  - global any-match flag via gpsimd partition_all_reduce -> register;
    fast path ships all -1 early; a predicated DMA rewrites on match
  - rare slow path (tc.If): recompute dots, build (-index / -4096)
    values, extract 64 smallest via vector.max + match_replace
"""

import numpy as np

import concourse.bacc as bacc
import concourse.mybir as mybir
from concourse.tile import TileContext
from concourse import bass_utils, bass_isa

B, L, D = 4, 2048, 64
HALF = 1024          # query rows per core
N_CORES = 8
K_MAX = 64
G = 32               # dims per group
QT = HALF // 128     # 8 query slabs per core
MATCH_DOT = 8.0      # 32 * 0.5 * 0.5
THRESH = 7.9         # between 7.75 (best non-match) and 8.0
SENT = 4096.0        # sentinel > any index

f32 = mybir.dt.float32
bf16 = mybir.dt.bfloat16
i32 = mybir.dt.int32
u32 = mybir.dt.uint32
Alu = mybir.AluOpType
Ax = mybir.AxisListType
AF = mybir.ActivationFunctionType

_CACHE = {}


def _build():
    nc = bacc.Bacc("TRN2", target_bir_lowering=False,
                   enable_partition_id=False)
    # qt4[h*64+d, pair*128+p] = q[p*8 + 2*pair + h, d]
    qt4 = nc.dram_tensor("qt4", [128, HALF // 2], f32, kind="ExternalInput")
    # kt4[dup*64+d, j] = k[j, d]
    kt4 = nc.dram_tensor("kt4", [128, L], f32, kind="ExternalInput")
    out = nc.dram_tensor("out", [HALF, K_MAX], i32, kind="ExternalOutput")
    out_pt = out[:].rearrange("(p t) c -> p (t c)", p=128)

    with TileContext(nc) as tc:
        with tc.tile_pool(name="sb", bufs=1) as sb, \
             tc.tile_pool(name="sb2", bufs=3) as sb2, \
             tc.tile_pool(name="ps", bufs=2, space="PSUM") as ps:

            # ---- load + sign-quantize (x>0 -> +0.5 else -0.5, bf16) ----
            qsb = sb.tile([128, HALF // 2], f32)
            ksb = sb.tile([128, L], f32)
            sqT4 = sb.tile([128, HALF // 2], bf16)
            skT4 = sb.tile([128, L], bf16)
            # first k quarter issued alone: its sign gates the ns=0
            # matmuls, so smaller transfer -> earlier pipeline start
            nc.default_dma_engine.dma_start(ksb[:, 0:512], kt4[:, 0:512])
            nc.scalar.dma_start(ksb[:, 1024:2048], kt4[:, 1024:2048])
            nc.default_dma_engine.dma_start(ksb[:, 512:1024],
                                            kt4[:, 512:1024])
            nc.default_dma_engine.dma_start(qsb, qt4[:, :])
            nc.vector.tensor_scalar(skT4[:, 0:512], ksb[:, 0:512],
                                    0.0, 0.5,
                                    op0=Alu.is_gt, op1=Alu.subtract)
            nc.vector.tensor_scalar(sqT4, qsb, 0.0, 0.5,
                                    op0=Alu.is_gt, op1=Alu.subtract)
            nc.vector.tensor_scalar(skT4[:, 512:1024], ksb[:, 512:1024],
                                    0.0, 0.5,
                                    op0=Alu.is_gt, op1=Alu.subtract)
            nc.vector.tensor_scalar(skT4[:, 1024:2048], ksb[:, 1024:2048],
                                    0.0, 0.5,
                                    op0=Alu.is_gt, op1=Alu.subtract)

            # ---- early fast-path output: all -1 (out_sb reused by the
            # slow path; WAR via the DMA read orders those writes after) --
            out_sb = sb.tile([128, QT * K_MAX], i32)
            nc.gpsimd.memset(out_sb, -1)
            nc.default_dma_engine.dma_start(out_pt, out_sb)

            # ---- all-pairs dots, 4 concurrent K=32 matmuls ----
            # rstat col semantics: >= THRESH iff any match (DVE row-max
            # cols top at 8; ACT relu-sum cols are 0 or >= 8)
            rstat = sb.tile([128, 32], f32)
            rbias = sb.tile([128, 1], f32)
            nc.vector.memset(rbias, -80.0 * THRESH)
            for pair in range(QT // 2):
                qc = slice(pair * 128, (pair + 1) * 128)
                for ns in range(4):
                    it = pair * 4 + ns
                    kc = slice(ns * 512, (ns + 1) * 512)
                    pG0 = ps.tile([128, 1024], f32, tag="g0")
                    pG1 = ps.tile([128, 1024], f32, tag="g1")
                    nc.tensor.matmul(pG0[:, 0:512], lhsT=sqT4[0:32, qc],
                                     rhs=skT4[0:32, kc], start=True,
                                     stop=True, tile_position=(0, 0))
                    nc.tensor.matmul(pG1[:, 0:512], lhsT=sqT4[32:64, qc],
                                     rhs=skT4[32:64, kc], start=True,
                                     stop=True, tile_position=(32, 0))
                    nc.tensor.matmul(pG0[:, 512:1024], lhsT=sqT4[64:96, qc],
                                     rhs=skT4[64:96, kc], start=True,
                                     stop=True, tile_position=(64, 0))
                    nc.tensor.matmul(pG1[:, 512:1024], lhsT=sqT4[96:128, qc],
                                     rhs=skT4[96:128, kc], start=True,
                                     stop=True, tile_position=(96, 0))
                    # detection: 17 units on DVE, 15 on ACT
                    for g, pg in ((0, pG0), (1, pG1)):
                        col = 2 * it + g
                        on_act = (g == 1) and it != 15
                        if on_act:
                            scr = sb2.tile([128, 1024], bf16, tag="scr")
                            nc.scalar.activation(
                                scr, pg, AF.Relu, bias=rbias[:, 0:1],
                                scale=80.0,
                                accum_out=rstat[:, col:col + 1])
                        else:
                            nc.vector.tensor_reduce(
                                rstat[:, col:col + 1], pg,
                                axis=Ax.X, op=Alu.max)

            # ---- scalar any-match flag ----
            ones = sb.tile([128, 1], f32)
            nc.vector.memset(ones, 1.0)
            sr = sb.tile([128, 1], f32)
            nc.vector.tensor_reduce(sr, rstat, axis=Ax.X, op=Alu.max)
            srf = sb.tile([128, 1], f32)
            nc.vector.tensor_scalar(srf, sr, THRESH, None, op0=Alu.is_ge)
            fps = ps.tile([1, 1], f32, tag="g0")
            nc.tensor.matmul(fps, lhsT=ones, rhs=srf, start=True, stop=True)
            flag = sb.tile([1, 1], i32)
            nc.vector.tensor_scalar(flag, fps[0:1, 0:1], 0.5, None,
                                    op0=Alu.is_ge)
            rv = nc.values_load(
                flag[0:1, 0:1], min_val=0, max_val=1,
                skip_runtime_bounds_check=True,
                engines=(mybir.EngineType.PE, mybir.EngineType.DVE,
                         mybir.EngineType.Pool, mybir.EngineType.SP))

            # ---- rare exact path ----
            with tc.If(rv > 0):
                c2i = sb.tile([128, L], i32)   # SENT - j (key j = column)
                nc.gpsimd.iota(c2i, pattern=[[-1, L]], base=int(SENT),
                               channel_multiplier=0)
                c2f = sb.tile([128, L], f32)
                nc.gpsimd.tensor_copy(c2f, c2i)
                negone = sb.tile([128, K_MAX], f32)
                nc.vector.memset(negone, -1.0)
                for t in range(QT):
                    base = (t % 2) * 64
                    qc = slice((t // 2) * 128, (t // 2) * 128 + 128)
                    lhs0 = sqT4[base:base + 32, qc]
                    lhs1 = sqT4[base + 32:base + 64, qc]
                    val = sb.tile([128, L], f32, tag="val")
                    for h in range(2):
                        p0 = ps.tile([128, 1024], f32, tag="g0")
                        p1 = ps.tile([128, 1024], f32, tag="g1")
                        for s in range(2):
                            kc = slice(h * 1024 + s * 512,
                                       h * 1024 + (s + 1) * 512)
                            sl = slice(s * 512, (s + 1) * 512)
                            nc.tensor.matmul(p0[:, sl], lhsT=lhs0,
                                             rhs=skT4[base:base + 32, kc],
                                             start=True, stop=True,
                                             tile_position=(base, 0))
                            nc.tensor.matmul(p1[:, sl], lhsT=lhs1,
                                             rhs=skT4[base + 32:base + 64,
                                                      kc],
                                             start=True, stop=True,
                                             tile_position=(base + 32, 0))
                        hsl = slice(h * 1024, (h + 1) * 1024)
                        m0 = sb2.tile([128, 1024], f32, tag="m0")
                        nc.vector.tensor_scalar(m0, p0, THRESH,
                                                None, op0=Alu.is_ge)
                        m1 = sb2.tile([128, 1024], f32, tag="m1")
                        nc.vector.scalar_tensor_tensor(
                            m1, in0=p1, scalar=THRESH, in1=m0,
                            op0=Alu.is_ge, op1=Alu.max)
                        # val = m1 ? -(j) : -SENT  ==  m1*(SENT-j) - SENT
                        nc.vector.tensor_tensor(
                            out=val[:, hsl], in0=m1, in1=c2f[:, hsl],
                            op=Alu.mult)
                        nc.vector.tensor_scalar_add(val[:, hsl], val[:, hsl],
                                                    -SENT)
                    # 64 smallest j == 64 largest of val, descending
                    no = sb.tile([128, K_MAX], f32, tag="no")
                    for it8 in range(8):
                        osl = slice(it8 * 8, (it8 + 1) * 8)
                        nc.vector.max(out=no[:, osl], in_=val)
                        nc.vector.match_replace(
                            out=val, in_to_replace=no[:, osl],
                            in_values=val, imm_value=-SENT)
                    jv = sb.tile([128, K_MAX], f32, tag="jv")
                    nc.vector.tensor_scalar_mul(jv, no, -1.0)  # j or SENT
                    msk = sb.tile([128, K_MAX], u32, tag="msk")
                    nc.vector.tensor_scalar(msk, jv, 2048.5, None,
                                            op0=Alu.is_ge)
                    nc.vector.copy_predicated(jv, msk, negone)
                    nc.vector.tensor_copy(
                        out_sb[:, t * K_MAX:(t + 1) * K_MAX], jv)

            # ---- predicated rewrite when a match exists ----
            nc.default_dma_engine.dma_start(out_pt, out_sb, cond=rv,
                                            cond_hint=False)

    nc.compile()
    return nc


def get_nc():
    if "nc" not in _CACHE:
        _CACHE["nc"] = _build()
    return _CACHE["nc"]


def make_in_maps(query_up, key_up):
    """Pure layout transforms (transpose/replicate/slice) per core."""
    query_up = np.asarray(query_up, dtype=np.float32)
    key_up = np.asarray(key_up, dtype=np.float32)
    in_maps = []
    for c in range(N_CORES):
        b, half = c // 2, c % 2
        q = query_up[b, half * HALF:(half + 1) * HALF]       # [1024, 64]
        # [p, pair, h, d] -> [h, d, pair, p] -> [128, 512]
        qt4 = np.ascontiguousarray(
            q.reshape(128, 4, 2, D).transpose(2, 3, 1, 0).reshape(
                128, HALF // 2))
        kT = key_up[b].T                                     # [64, 2048]
        kt4 = np.ascontiguousarray(np.concatenate([kT, kT], axis=0))
        in_maps.append({"qt4": qt4, "kt4": kt4})
    return in_maps


def kernel(query_up, key_up, head_idx=None, **_ignored):
    nc = get_nc()
    in_maps = make_in_maps(query_up, key_up)
    res = bass_utils.run_bass_kernel_spmd(
        nc, in_maps, core_ids=list(range(N_CORES)))
    full = np.empty((B, L, K_MAX), dtype=np.int32)
    for c in range(N_CORES):
        b, half = c // 2, c % 2
        # out row p*8 + t <-> query row p*8 + 2*(t//2) + t%2 == p*8 + t
        full[b, half * HALF:(half + 1) * HALF] = res.results[c]["out"]
    return full

